# revision 31
# baseline (speedup 1.0000x reference)
"""Trainium2 Bass kernel for nn_MultiHeadAttention_84318797955257.

Inverted-softmax multi-head attention, 8-core SPMD:
  - Sharding: 4 batches x 2 T-halves (each core: 1 batch, 512 query rows,
    all 16 heads, full S).  The inverted softmax denominator Z[b,s] (sum of
    exp over heads*T) is the only cross-core quantity: a tiny [128,8] f32
    AllReduce over core pairs.  Final S-renorm and the output projection are
    fully core-local (output rows are disjoint across cores).
  - Mask folded into bias host-side as -30000 additive (exp underflows to 0,
    matching the reference's where(mask,0) exactly).
  - Scores are computed in [s, t] layout (eT = khT.T @ qhT per head) so the
    AV matmul needs no transpose.  Bias is accumulated into the scores PSUM
    with an identity matmul (PE), keeping DVE off the critical path.
  - exp runs on ACT with accum_out giving the Z partial sums for free.
  - 1/Z is folded into V rows (v' = v/Z) instead of scaling the 8.4M score
    weights; an extra ones-column in V yields the S-renorm denominator r.
  - 1/(r+eps) via ACT log/exp (ACT Reciprocal is banned); broadcast across
    partitions with a K=1 outer-product matmul.
"""

import numpy as np
import ml_dtypes
import bass_rust
import concourse.bass as bass
import concourse.mybir as mybir
import concourse.tile as tile
from concourse.bass_utils import run_bass_kernel_spmd
from concourse.vector_clock import ScopedClock
from concourse.masks import make_identity

AF = mybir.ActivationFunctionType
ALU = mybir.AluOpType
f32 = mybir.dt.float32
f16 = mybir.dt.float16
bf16 = mybir.dt.bfloat16

B, T, S, D, H, DH = 4, 1024, 1024, 1024, 16, 64
P = 128
TL = T // 2          # 512 query rows per core
NEG = -30000.0
EPS = 1e-5
NCORES = 8
REPLICA_GROUPS = [[0, 1], [2, 3], [4, 5], [6, 7]]


# ---------------------------------------------------------------------------
# Workaround: this container's walrus build allows only ONE sync-wait per
# instruction ("Too many sync wait commands" in setupSyncWait).  After Tile
# scheduling, split any instruction's extra waits onto preceding NOPs on the
# same engine (the engine sequencer blocks on each wait in program order, so
# semantics are identical).
# ---------------------------------------------------------------------------
def _split_multi_waits(nc):
    cnt = 0
    for fn in nc.m.functions:
        for bb in fn.blocks:
            new = []
            changed = False
            for inst in bb.instructions:
                si = inst.sync_info
                if si is not None and len(si.on_wait) > 1:
                    changed = True
                    waits = list(si.on_wait)
                    for w in waits[:-1]:
                        cnt += 1
                        nop = mybir.InstNoOp(
                            name=f"I-waitsplit-{cnt}", ins=[], outs=[]
                        )
                        nop.engine = inst.engine
                        nop.sync_info = bass_rust.SyncInfo(
                            on_wait=[w], on_update=[]
                        )
                        new.append(nop)
                    inst.sync_info = bass_rust.SyncInfo(
                        on_wait=[waits[-1]], on_update=list(si.on_update)
                    )
                new.append(inst)
            if changed:
                bb.instructions = new
    return cnt


# ---------------------------------------------------------------------------
# Device program (identical on all 8 cores)
# ---------------------------------------------------------------------------
def _emit(nc, tc, qT, kT, vT, WqT, WkT, WvT, WoT, biasT, out):
    from contextlib import ExitStack

    with ExitStack() as ctx:
        perst = ctx.enter_context(tc.tile_pool(name="perst", bufs=1))
        ident = perst.tile([P, P], bf16)
        make_identity(nc, ident[:])
        zball2 = perst.tile([P, 64, 2], f32)  # per (s_tile, head-pair) Z
        zloc = perst.tile([P, 8], f32)
        zg = perst.tile([P, 8], f32)
        zinv = perst.tile([P, 8], f32)
        ones_col = perst.tile([P, 16], f32)
        nc.gpsimd.memset(ones_col[:], 1.0)
        ones1 = perst.tile([P, 64], f16)   # row 64 used (must match rhs base)
        nc.gpsimd.memset(ones1[:], 1.0)
        eps_t = perst.tile([P, 1], f32)
        nc.gpsimd.memset(eps_t[:], EPS)

        # exp(scores) for all (head, s_tile): [s_loc, h*8+s_tile, t], f16
        wbuf_pool = ctx.enter_context(tc.tile_pool(name="wbufp", bufs=1))
        wbuf = wbuf_pool.tile([P, 128, TL], f16)

        vaug_pool = ctx.enter_context(tc.tile_pool(name="vaugp", bufs=1))
        vaug = [vaug_pool.tile([P, 16, 65], f16, name=f"vaug{i}")
                for i in range(8)]

        qk_stack = ctx.enter_context(ExitStack())
        qk_pool = qk_stack.enter_context(tc.tile_pool(name="qk", bufs=1))
        qh = qk_pool.tile([P, 8, TL], f16)      # qhT: [f_loc, f_tile, t]
        kh = qk_pool.tile([P, 8, S], f16)       # khT: [f_loc, f_tile, s]

        # ---- phase A: q/k projections.  One LDWEIGHTS feeds two matmuls
        # (split-N for q, the two s-blocks for k) so consecutive matmuls
        # share the stationary operand and pipeline on the PE. ----
        with ExitStack() as actx:
            apool = actx.enter_context(tc.tile_pool(name="apool", bufs=1))
            apsum = actx.enter_context(
                tc.tile_pool(name="apsum", bufs=2, space="PSUM")
            )
            wqr = WqT[:].rearrange("(o p) f -> p o f", p=P)
            with tc.tile_pool(name="qtp", bufs=1) as qtp:
                qt = qtp.tile([P, 8, TL], f16)
                qtr_ = qT[:].rearrange("(o p) t -> p o t", p=P)
                for ch in range(2):
                    nc.sync.dma_start(
                        qt[:, ch * 4:(ch + 1) * 4, :],
                        qtr_[:, ch * 4:(ch + 1) * 4, :],
                    )
                for fh in range(2):
                    wq = apool.tile([P, 8, 512], f16, tag="w")
                    nc.sync.dma_start(
                        wq[:], wqr[:, :, fh * 512:(fh + 1) * 512]
                    )
                    for fl in range(4):
                        f = fh * 4 + fl
                        ps = apsum.tile([P, 512], f32, tag="qps")
                        for c in range(8):
                            lw = wq[:, c, fl * P:(fl + 1) * P]
                            nc.tensor.matmul(
                                ps[:, 0:256], lw, qt[:, c, 0:256],
                                start=(c == 0), stop=False,
                            )
                            nc.tensor.matmul(
                                ps[:, 256:512], lw, qt[:, c, 256:512],
                                start=False, stop=(c == 7),
                            )
                        nc.vector.tensor_scalar_mul(
                            qh[:, f, :], ps[:], DH ** -0.5
                        )

            wkr = WkT[:].rearrange("(o p) f -> p o f", p=P)
            with tc.tile_pool(name="ktp", bufs=1) as ktp:
                kt = ktp.tile([P, 8, S], f16)
                ktr_ = kT[:].rearrange("(o p) s -> p o s", p=P)
                for ch in range(4):
                    nc.sync.dma_start(
                        kt[:, ch * 2:(ch + 1) * 2, :],
                        ktr_[:, ch * 2:(ch + 1) * 2, :],
                    )
                for fh in range(2):
                    wk = apool.tile([P, 8, 512], f16, tag="w")
                    nc.sync.dma_start(
                        wk[:], wkr[:, :, fh * 512:(fh + 1) * 512]
                    )
                    for fl in range(4):
                        f = fh * 4 + fl
                        psp = apsum.tile([P, 2, 512], f32, tag="kps")
                        for c in range(8):
                            lw = wk[:, c, fl * P:(fl + 1) * P]
                            nc.tensor.matmul(
                                psp[:, 0, :], lw, kt[:, c, 0:512],
                                start=(c == 0), stop=(c == 7),
                            )
                            nc.tensor.matmul(
                                psp[:, 1, :], lw, kt[:, c, 512:1024],
                                start=(c == 0), stop=(c == 7),
                            )
                        nc.vector.tensor_copy(
                            kh[:, f, :].rearrange("p (b t) -> p b t", b=2),
                            psp[:],
                        )

        # C-phase pools opened alongside phase B so the v-projection matmuls
        # (independent of B) can fill PE idle slots while ACT runs the exps.
        c_stack = ctx.enter_context(ExitStack())
        vpool = c_stack.enter_context(tc.tile_pool(name="vpool", bufs=1))
        wvpool = c_stack.enter_context(tc.tile_pool(name="wvpool", bufs=1))
        cpsum = c_stack.enter_context(
            tc.tile_pool(name="cpsum", bufs=2, space="PSUM")
        )
        vtr = vT[:].rearrange("(o p) s -> p o s", p=P)
        wv = wvpool.tile([P, 8, S], f16)
        wvr_ = WvT[:].rearrange("(o p) f -> p o f", p=P)
        for ch in range(4):
            nc.sync.dma_start(
                wv[:, ch * 2:(ch + 1) * 2, :], wvr_[:, ch * 2:(ch + 1) * 2, :]
            )
        drp = ctx.enter_context(tc.tile_pool(name="drp", bufs=1, space="DRAM"))
        din = [drp.tile([P, 4], f32, name=f"din{s}") for s in range(2)]
        dout = [drp.tile([P, 4], f32, name=f"dout{s}") for s in range(2)]

        # ---- phases B+C chunked by s-half, with a split Z-allreduce -------
        with ExitStack() as bctx:
            biasp = bctx.enter_context(tc.tile_pool(name="biasp", bufs=2))
            bpsum = bctx.enter_context(
                tc.tile_pool(name="bpsum", bufs=2, space="PSUM")
            )
            for sh in range(2):
                for i in range(sh * 4, sh * 4 + 4):
                    for qtr in range(4):
                        bt = biasp.tile([P, 4, TL], bf16, tag="bias")
                        nc.sync.dma_start(bt[:], biasT[i, qtr])
                        pss = []
                        for hp in range(2):   # head pairs within the quarter
                            ps = bpsum.tile([P, 2, 512], f32, tag="eps")
                            pss.append(ps)
                            for u in range(2):
                                h = qtr * 4 + hp * 2 + u
                                po = (h % 2) * 64
                                lw = kh[po:po + 64, h // 2,
                                        i * P:(i + 1) * P]
                                rq = qh[po:po + 64, h // 2, :]
                                nc.tensor.matmul(
                                    ps[:, u, 0:256], lw, rq[:, 0:256],
                                    start=True, stop=False,
                                )
                                nc.tensor.matmul(
                                    ps[:, u, 256:512], lw, rq[:, 256:512],
                                    start=False, stop=False,
                                )
                        for hp in range(2):
                            for u in range(2):
                                nc.tensor.matmul(
                                    pss[hp][:, u, :], ident[:],
                                    bt[:, hp * 2 + u, :],
                                    start=False, stop=True,
                                )
                        for hp in range(2):
                            h0 = qtr * 4 + hp * 2
                            wpair = wbuf[:].rearrange(
                                "p (h i) t -> p h i t", i=8
                            )[:, h0:h0 + 2, i, :]
                            nc.scalar.activation(wpair, pss[hp][:], AF.Exp)
                            nc.vector.tensor_reduce(
                                zball2[:, i * 8 + qtr * 2 + hp, :],
                                wpair,
                                axis=mybir.AxisListType.X, op=ALU.add,
                            )
                    nc.vector.tensor_reduce(
                        zloc[:, i:i + 1],
                        zball2[:, i * 8:(i + 1) * 8, :],
                        axis=mybir.AxisListType.XY, op=ALU.add,
                    )

                # C chunk for this s-half (PE filler during B's exp stalls)
                vt = vpool.tile([P, 8, 512], f16, tag="vt")
                nc.sync.dma_start(vt[:], vtr[:, :, sh * 512:(sh + 1) * 512])
                for il in range(4):
                    i = sh * 4 + il
                    psp = cpsum.tile([P, 2, 512], f32, tag="vps")
                    for c in range(8):
                        lw = vt[:, c, il * P:(il + 1) * P]
                        nc.tensor.matmul(
                            psp[:, 0, :], lw, wv[:, c, 0:512],
                            start=(c == 0), stop=(c == 7),
                        )
                        nc.tensor.matmul(
                            psp[:, 1, :], lw, wv[:, c, 512:1024],
                            start=(c == 0), stop=(c == 7),
                        )
                    nc.vector.tensor_copy(
                        vaug[i][:, :, 0:64],
                        psp[:].rearrange("p b (h c) -> p (b h) c", c=64),
                    )

                # AllReduce this half's Z over the core pair
                nc.gpsimd.dma_start(din[sh][:], zloc[:, sh * 4:(sh + 1) * 4])
                nc.gpsimd.collective_compute(
                    "AllReduce", ALU.add, replica_groups=REPLICA_GROUPS,
                    ins=[din[sh].opt()], outs=[dout[sh].opt()],
                )
                nc.gpsimd.dma_start(zg[:, sh * 4:(sh + 1) * 4], dout[sh][:])
                nc.vector.reciprocal(zinv[:, sh * 4:(sh + 1) * 4],
                                     zg[:, sh * 4:(sh + 1) * 4])
                for i in range(sh * 4, sh * 4 + 4):
                    nc.vector.tensor_scalar_mul(
                        vaug[i][:, :, 0:64], vaug[i][:, :, 0:64],
                        zinv[:, i:i + 1],
                    )
                    nc.vector.tensor_scalar_mul(
                        vaug[i][:, :, 64:65], ones_col[:, :, None],
                        zinv[:, i:i + 1],
                    )

        c_stack.close()
        qk_stack.close()   # qh/kh no longer needed

        # ---- phase D: AV + renorm;  phase E: output projection ------------
        aopool = ctx.enter_context(tc.tile_pool(name="aop", bufs=1))
        aop = [aopool.tile([P, 512], f16, name=f"aop{j}") for j in range(8)]
        with ExitStack() as dctx:
            dpool = dctx.enter_context(tc.tile_pool(name="dpool", bufs=2))
            epool = dctx.enter_context(tc.tile_pool(name="epool", bufs=2))
            dpsum = dctx.enter_context(
                tc.tile_pool(name="dpsum", bufs=2, space="PSUM")
            )
            bpsum2 = dctx.enter_context(
                tc.tile_pool(name="bpsum2", bufs=2, space="PSUM")
            )
            opsum = dctx.enter_context(
                tc.tile_pool(name="opsum", bufs=1, space="PSUM")
            )
            wopool = dctx.enter_context(tc.tile_pool(name="wop", bufs=1))
            # WoT rows packed [d, j, dout] per head-PAIR (d = 128 = two
            # heads' features) so lhsT/rhs are full-K 128-partition tiles.
            wo = wopool.tile([P, 8, D], f16)
            wor_ = WoT[:].rearrange("(j d) o -> d j o", d=128)
            for ch in range(4):
                nc.sync.dma_start(
                    wo[:, ch * 2:(ch + 1) * 2, :],
                    wor_[:, ch * 2:(ch + 1) * 2, :],
                )
            tmpA = [dpool.tile([P, 512], f16, name=f"tmpA{h}", bufs=1)
                    for h in range(16)]
            # D1: s-tiles 0-3 chains for ALL heads (independent of the second
            # Z allreduce) — keeps PE busy through the AR window.
            for h in range(16):
                psA = dpsum.tile([P, 512], f32, tag="avpA")
                for i in range(4):
                    lw = vaug[i][:, h, :]
                    rw = wbuf[:, h * 8 + i, :]
                    nc.tensor.matmul(
                        psA[0:65, 0:256], lw, rw[:, 0:256],
                        start=(i == 0), stop=False,
                    )
                    nc.tensor.matmul(
                        psA[0:65, 256:512], lw, rw[:, 256:512],
                        start=False, stop=(i == 3),
                    )
                nc.scalar.copy(tmpA[h][0:65, :], psA[0:65, :])
            # D2: s-tiles 4-7 chains, merge, renorm
            for h in range(16):
                psB = dpsum.tile([P, 512], f32, tag="avpB")
                for i in range(4, 8):
                    lw = vaug[i][:, h, :]
                    rw = wbuf[:, h * 8 + i, :]
                    nc.tensor.matmul(
                        psB[0:65, 0:256], lw, rw[:, 0:256],
                        start=(i == 4), stop=False,
                    )
                    nc.tensor.matmul(
                        psB[0:65, 256:512], lw, rw[:, 256:512],
                        start=False, stop=(i == 7),
                    )
                tmp = dpool.tile([P, 512], f32, tag="rtmp")
                nc.vector.tensor_add(
                    out=tmp[0:65, :], in0=psB[0:65, :], in1=tmpA[h][0:65, :]
                )
                nc.scalar.activation(
                    tmp[64:65, :], tmp[64:65, :], AF.Ln,
                    bias=eps_t[64:65, :],
                )
                rinv = dpool.tile([P, 512], f16, tag="rinv")
                nc.scalar.activation(
                    rinv[64:65, :], tmp[64:65, :], AF.Exp,
                    scale=-1.0,
                )
                pb = bpsum2.tile([64, 512], f32, tag="bcp")
                nc.tensor.matmul(
                    pb[:], ones1[64:65, :], rinv[64:65, :],
                    start=True, stop=True,
                )
                bc = dpool.tile([64, 512], f32, tag="bc")
                nc.vector.tensor_copy(bc[:], pb[:])
                po = (h % 2) * 64
                nc.vector.tensor_mul(
                    out=aop[h // 2][po:po + 64, :],
                    in0=tmp[0:64, :], in1=bc[:],
                )

            # ---- phase E: output projection (interleaves with D).  One
            # LDWEIGHTS per (tch, j) feeds both dout halves. ----
            for tch in range(4):
                pso = opsum.tile([P, 2, 512], f32, tag="outp")
                for j in range(8):
                    lw = aop[j][:, tch * P:(tch + 1) * P]
                    nc.tensor.matmul(
                        pso[:, 0, :], lw, wo[:, j, 0:512],
                        start=(j == 0), stop=(j == 7),
                    )
                    nc.tensor.matmul(
                        pso[:, 1, :], lw, wo[:, j, 512:1024],
                        start=(j == 0), stop=(j == 7),
                    )
                ot = epool.tile([P, 2, 512], f32, tag="ot")
                nc.vector.tensor_copy(ot[:], pso[:])
                nc.sync.dma_start(
                    out[tch * P:(tch + 1) * P, :],
                    ot[:].rearrange("p a b -> p (a b)"),
                )


def build_nc():
    nc = bass.Bass(num_devices=NCORES)
    qT = nc.dram_tensor("qT", [D, TL], f16, kind="ExternalInput")
    kT = nc.dram_tensor("kT", [D, S], f16, kind="ExternalInput")
    vT = nc.dram_tensor("vT", [D, S], f16, kind="ExternalInput")
    WqT = nc.dram_tensor("WqT", [D, D], f16, kind="ExternalInput")
    WkT = nc.dram_tensor("WkT", [D, D], f16, kind="ExternalInput")
    WvT = nc.dram_tensor("WvT", [D, D], f16, kind="ExternalInput")
    WoT = nc.dram_tensor("WoT", [D, D], f16, kind="ExternalInput")
    biasT = nc.dram_tensor("biasT", [8, 4, P, 4, TL], bf16, kind="ExternalInput")
    out = nc.dram_tensor("out", [TL, D], f32, kind="ExternalOutput")
    with tile.TileContext(nc) as tc:
        _emit(nc, tc, qT, kT, vT, WqT, WkT, WvT, WoT, biasT, out)
    _split_multi_waits(nc)
    return nc


_NC_CACHE = {}


def _get_nc():
    if "nc" not in _NC_CACHE:
        _NC_CACHE["nc"] = build_nc()
    return _NC_CACHE["nc"]


# ---------------------------------------------------------------------------
# Host wrapper
# ---------------------------------------------------------------------------
def _prep_inputs(q, k, v, attn_mask, attn_bias, Wq, Wk, Wv, Wo):
    q = np.asarray(q, np.float32)
    k = np.asarray(k, np.float32)
    v = np.asarray(v, np.float32)
    mask = np.asarray(attn_mask)
    bias = np.asarray(attn_bias, np.float32)
    WqT = np.ascontiguousarray(np.asarray(Wq, np.float32).T).astype(np.float16)
    WkT = np.ascontiguousarray(np.asarray(Wk, np.float32).T).astype(np.float16)
    WvT = np.ascontiguousarray(np.asarray(Wv, np.float32).T).astype(np.float16)
    WoT = np.ascontiguousarray(np.asarray(Wo, np.float32).T).astype(np.float16)

    # bias + additive mask, transposed to [h, s, t], tiled for DMA:
    # [s_tile, half, p, head_in_half, t]  with h = half*8+hh, s = s_tile*128+p
    pre = bias + np.where(mask, np.float32(NEG), np.float32(0.0))[None]
    biasT_halves = []
    for th in range(2):
        arrT = pre[:, th * TL:(th + 1) * TL, :].transpose(0, 2, 1)  # [h,s,t]
        arrT = np.ascontiguousarray(arrT).reshape(4, 4, 8, P, TL)
        arrT = arrT.transpose(2, 0, 3, 1, 4)  # [i, qtr, p, hh, t]
        biasT_halves.append(
            np.ascontiguousarray(arrT).astype(ml_dtypes.bfloat16)
        )

    in_maps = []
    kTs, vTs = {}, {}
    for c in range(NCORES):
        b, th = c // 2, c % 2
        if b not in kTs:
            kTs[b] = np.ascontiguousarray(k[b].T).astype(np.float16)
            vTs[b] = np.ascontiguousarray(v[b].T).astype(np.float16)
        qTc = np.ascontiguousarray(
            q[b].T[:, th * TL:(th + 1) * TL]).astype(np.float16)
        in_maps.append({
            "qT": qTc, "kT": kTs[b], "vT": vTs[b],
            "WqT": WqT, "WkT": WkT, "WvT": WvT, "WoT": WoT,
            "biasT": biasT_halves[th],
        })
    return in_maps


def run(inputs, trace=False):
    nc = _get_nc()
    in_maps = _prep_inputs(**inputs)
    res = run_bass_kernel_spmd(
        nc, in_maps, core_ids=list(range(NCORES)), trace=trace,
    )
    full = np.empty((B, T, D), np.float32)
    for c in range(NCORES):
        b, th = c // 2, c % 2
        full[b, th * TL:(th + 1) * TL, :] = res.results[c]["out"]
    return full, res


def kernel(**inputs) -> np.ndarray:
    full, _ = run(inputs, trace=False)
    return full


# revision 32
# speedup vs baseline: 1.2632x; 1.2632x over previous
"""Trainium2 Bass kernel for nn_MultiHeadAttention_84318797955257.

Inverted-softmax multi-head attention, 8-core SPMD:
  - Sharding: 4 batches x 2 T-halves (each core: 1 batch, 512 query rows,
    all 16 heads, full S).  The inverted softmax denominator Z[b,s] (sum of
    exp over heads*T) is the only cross-core quantity: a tiny [128,8] f32
    AllReduce over core pairs.  Final S-renorm and the output projection are
    fully core-local (output rows are disjoint across cores).
  - Mask folded into bias host-side as -30000 additive (exp underflows to 0,
    matching the reference's where(mask,0) exactly).
  - Scores are computed in [s, t] layout (eT = khT.T @ qhT per head) so the
    AV matmul needs no transpose.  Bias is accumulated into the scores PSUM
    with an identity matmul (PE), keeping DVE off the critical path.
  - exp runs on ACT with accum_out giving the Z partial sums for free.
  - 1/Z is folded into V rows (v' = v/Z) instead of scaling the 8.4M score
    weights; an extra ones-column in V yields the S-renorm denominator r.
  - 1/(r+eps) via ACT log/exp (ACT Reciprocal is banned); broadcast across
    partitions with a K=1 outer-product matmul.
"""

import numpy as np
import ml_dtypes
import bass_rust
import concourse.bass as bass
import concourse.mybir as mybir
import concourse.tile as tile
from concourse.bass_utils import run_bass_kernel_spmd
from concourse.vector_clock import ScopedClock
from concourse.masks import make_identity

AF = mybir.ActivationFunctionType
ALU = mybir.AluOpType
f32 = mybir.dt.float32
f16 = mybir.dt.float16
bf16 = mybir.dt.bfloat16

B, T, S, D, H, DH = 4, 1024, 1024, 1024, 16, 64
P = 128
TL = T // 2          # 512 query rows per core
NEG = -30000.0
EPS = 1e-5
NCORES = 8
REPLICA_GROUPS = [[0, 1], [2, 3], [4, 5], [6, 7]]


# ---------------------------------------------------------------------------
# Workaround: this container's walrus build allows only ONE sync-wait per
# instruction ("Too many sync wait commands" in setupSyncWait).  After Tile
# scheduling, split any instruction's extra waits onto preceding NOPs on the
# same engine (the engine sequencer blocks on each wait in program order, so
# semantics are identical).
# ---------------------------------------------------------------------------
def _split_multi_waits(nc):
    cnt = 0
    for fn in nc.m.functions:
        for bb in fn.blocks:
            new = []
            changed = False
            for inst in bb.instructions:
                si = inst.sync_info
                if si is not None and len(si.on_wait) > 1:
                    changed = True
                    waits = list(si.on_wait)
                    for w in waits[:-1]:
                        cnt += 1
                        nop = mybir.InstNoOp(
                            name=f"I-waitsplit-{cnt}", ins=[], outs=[]
                        )
                        nop.engine = inst.engine
                        nop.sync_info = bass_rust.SyncInfo(
                            on_wait=[w], on_update=[]
                        )
                        new.append(nop)
                    inst.sync_info = bass_rust.SyncInfo(
                        on_wait=[waits[-1]], on_update=list(si.on_update)
                    )
                new.append(inst)
            if changed:
                bb.instructions = new
    return cnt


# ---------------------------------------------------------------------------
# Device program (identical on all 8 cores)
# ---------------------------------------------------------------------------
def _emit(nc, tc, qT, kT, vT, WqT, WkT, WvT, WoT, biasT, out):
    from contextlib import ExitStack

    with ExitStack() as ctx:
        perst = ctx.enter_context(tc.tile_pool(name="perst", bufs=1))
        ident = perst.tile([P, P], bf16)
        make_identity(nc, ident[:])
        zball2 = perst.tile([P, 64, 2], f32)  # per (s_tile, head-pair) Z
        zloc = perst.tile([P, 8], f32)
        zg = perst.tile([P, 8], f32)
        zinv = perst.tile([P, 8], f32)
        ones_col = perst.tile([P, 16], f32)
        nc.gpsimd.memset(ones_col[:], 1.0)
        ones1 = perst.tile([P, 64], f16)   # row 64 used (must match rhs base)
        nc.gpsimd.memset(ones1[:], 1.0)
        eps_t = perst.tile([P, 1], f32)
        nc.gpsimd.memset(eps_t[:], EPS)

        # exp(scores) for all (head, s_tile): [s_loc, h*8+s_tile, t], f16
        wbuf_pool = ctx.enter_context(tc.tile_pool(name="wbufp", bufs=1))
        wbuf = wbuf_pool.tile([P, 128, TL], f16)

        vaug_pool = ctx.enter_context(tc.tile_pool(name="vaugp", bufs=1))
        vaug = [vaug_pool.tile([P, 16, 65], f16, name=f"vaug{i}")
                for i in range(8)]

        qk_stack = ctx.enter_context(ExitStack())
        qk_pool = qk_stack.enter_context(tc.tile_pool(name="qk", bufs=1))
        qh = qk_pool.tile([P, 8, TL], f16)      # qhT: [f_loc, f_tile, t]
        kh = qk_pool.tile([P, 8, S], f16)       # khT: [f_loc, f_tile, s]

        # ---- phase A: q/k projections.  One LDWEIGHTS feeds two matmuls
        # (split-N for q, the two s-blocks for k) so consecutive matmuls
        # share the stationary operand and pipeline on the PE. ----
        with ExitStack() as actx:
            apool = actx.enter_context(tc.tile_pool(name="apool", bufs=1))
            apsum = actx.enter_context(
                tc.tile_pool(name="apsum", bufs=2, space="PSUM")
            )
            wqr = WqT[:].rearrange("(o p) f -> p o f", p=P)
            with tc.tile_pool(name="qtp", bufs=1) as qtp:
                qt = qtp.tile([P, 8, TL], f16)
                qtr_ = qT[:].rearrange("(o p) t -> p o t", p=P)
                for ch in range(2):
                    nc.sync.dma_start(
                        qt[:, ch * 4:(ch + 1) * 4, :],
                        qtr_[:, ch * 4:(ch + 1) * 4, :],
                    )
                for fh in range(2):
                    wq = apool.tile([P, 8, 512], f16, tag="w")
                    nc.sync.dma_start(
                        wq[:], wqr[:, :, fh * 512:(fh + 1) * 512]
                    )
                    for fl in range(4):
                        f = fh * 4 + fl
                        ps = apsum.tile([P, 512], f32, tag="qps")
                        for c in range(8):
                            nc.tensor.matmul(
                                ps[:], wq[:, c, fl * P:(fl + 1) * P],
                                qt[:, c, :],
                                start=(c == 0), stop=(c == 7),
                            )
                        nc.vector.tensor_scalar_mul(
                            qh[:, f, :], ps[:], DH ** -0.5
                        )

            wkr = WkT[:].rearrange("(o p) f -> p o f", p=P)
            with tc.tile_pool(name="ktp", bufs=1) as ktp:
                kt = ktp.tile([P, 8, S], f16)
                ktr_ = kT[:].rearrange("(o p) s -> p o s", p=P)
                for ch in range(4):
                    nc.sync.dma_start(
                        kt[:, ch * 2:(ch + 1) * 2, :],
                        ktr_[:, ch * 2:(ch + 1) * 2, :],
                    )
                for fh in range(2):
                    wk = apool.tile([P, 8, 512], f16, tag="w")
                    nc.sync.dma_start(
                        wk[:], wkr[:, :, fh * 512:(fh + 1) * 512]
                    )
                    for fl in range(4):
                        f = fh * 4 + fl
                        psp = apsum.tile([P, 2, 512], f32, tag="kps")
                        for c in range(8):
                            lw = wk[:, c, fl * P:(fl + 1) * P]
                            nc.tensor.matmul(
                                psp[:, 0, :], lw, kt[:, c, 0:512],
                                start=(c == 0), stop=(c == 7),
                            )
                            nc.tensor.matmul(
                                psp[:, 1, :], lw, kt[:, c, 512:1024],
                                start=(c == 0), stop=(c == 7),
                            )
                        nc.vector.tensor_copy(
                            kh[:, f, :].rearrange("p (b t) -> p b t", b=2),
                            psp[:],
                        )

        # C-phase pools opened alongside phase B so the v-projection matmuls
        # (independent of B) can fill PE idle slots while ACT runs the exps.
        c_stack = ctx.enter_context(ExitStack())
        vpool = c_stack.enter_context(tc.tile_pool(name="vpool", bufs=1))
        wvpool = c_stack.enter_context(tc.tile_pool(name="wvpool", bufs=1))
        cpsum = c_stack.enter_context(
            tc.tile_pool(name="cpsum", bufs=2, space="PSUM")
        )
        vtr = vT[:].rearrange("(o p) s -> p o s", p=P)
        wv = wvpool.tile([P, 8, S], f16)
        wvr_ = WvT[:].rearrange("(o p) f -> p o f", p=P)
        for ch in range(4):
            nc.sync.dma_start(
                wv[:, ch * 2:(ch + 1) * 2, :], wvr_[:, ch * 2:(ch + 1) * 2, :]
            )
        drp = ctx.enter_context(tc.tile_pool(name="drp", bufs=1, space="DRAM"))
        din = [drp.tile([P, 4], f32, name=f"din{s}") for s in range(2)]
        dout = [drp.tile([P, 4], f32, name=f"dout{s}") for s in range(2)]

        # ---- phases B+C chunked by s-half, with a split Z-allreduce -------
        with ExitStack() as bctx:
            biasp = bctx.enter_context(tc.tile_pool(name="biasp", bufs=2))
            bpsum = bctx.enter_context(
                tc.tile_pool(name="bpsum", bufs=2, space="PSUM")
            )
            for sh in range(2):
                for i in range(sh * 4, sh * 4 + 4):
                    for qtr in range(4):
                        bt = biasp.tile([P, 4, TL], bf16, tag="bias")
                        nc.sync.dma_start(bt[:], biasT[i, qtr])
                        pss = []
                        for hp in range(2):   # head pairs within the quarter
                            ps = bpsum.tile([P, 2, 512], f32, tag="eps")
                            pss.append(ps)
                            for u in range(2):
                                h = qtr * 4 + hp * 2 + u
                                po = (h % 2) * 64
                                lw = kh[po:po + 64, h // 2,
                                        i * P:(i + 1) * P]
                                rq = qh[po:po + 64, h // 2, :]
                                nc.tensor.matmul(
                                    ps[:, u, :], lw, rq,
                                    start=True, stop=False,
                                )
                        for hp in range(2):
                            for u in range(2):
                                nc.tensor.matmul(
                                    pss[hp][:, u, :], ident[:],
                                    bt[:, hp * 2 + u, :],
                                    start=False, stop=True,
                                )
                        for hp in range(2):
                            h0 = qtr * 4 + hp * 2
                            wpair = wbuf[:].rearrange(
                                "p (h i) t -> p h i t", i=8
                            )[:, h0:h0 + 2, i, :]
                            nc.scalar.activation(wpair, pss[hp][:], AF.Exp)
                            nc.vector.tensor_reduce(
                                zball2[:, i * 8 + qtr * 2 + hp, :],
                                wpair,
                                axis=mybir.AxisListType.X, op=ALU.add,
                            )
                    nc.vector.tensor_reduce(
                        zloc[:, i:i + 1],
                        zball2[:, i * 8:(i + 1) * 8, :],
                        axis=mybir.AxisListType.XY, op=ALU.add,
                    )

                # C chunk for this s-half (PE filler during B's exp stalls)
                vt = vpool.tile([P, 8, 512], f16, tag="vt")
                nc.sync.dma_start(vt[:], vtr[:, :, sh * 512:(sh + 1) * 512])
                for il in range(4):
                    i = sh * 4 + il
                    psp = cpsum.tile([P, 2, 512], f32, tag="vps")
                    for c in range(8):
                        lw = vt[:, c, il * P:(il + 1) * P]
                        nc.tensor.matmul(
                            psp[:, 0, :], lw, wv[:, c, 0:512],
                            start=(c == 0), stop=(c == 7),
                        )
                        nc.tensor.matmul(
                            psp[:, 1, :], lw, wv[:, c, 512:1024],
                            start=(c == 0), stop=(c == 7),
                        )
                    nc.vector.tensor_copy(
                        vaug[i][:, :, 0:64],
                        psp[:].rearrange("p b (h c) -> p (b h) c", c=64),
                    )

                # AllReduce this half's Z over the core pair
                nc.gpsimd.dma_start(din[sh][:], zloc[:, sh * 4:(sh + 1) * 4])
                nc.gpsimd.collective_compute(
                    "AllReduce", ALU.add, replica_groups=REPLICA_GROUPS,
                    ins=[din[sh].opt()], outs=[dout[sh].opt()],
                )
                nc.gpsimd.dma_start(zg[:, sh * 4:(sh + 1) * 4], dout[sh][:])
                nc.vector.reciprocal(zinv[:, sh * 4:(sh + 1) * 4],
                                     zg[:, sh * 4:(sh + 1) * 4])
                for i in range(sh * 4, sh * 4 + 4):
                    nc.vector.tensor_scalar_mul(
                        vaug[i][:, :, 0:64], vaug[i][:, :, 0:64],
                        zinv[:, i:i + 1],
                    )
                    nc.vector.tensor_scalar_mul(
                        vaug[i][:, :, 64:65], ones_col[:, :, None],
                        zinv[:, i:i + 1],
                    )

        c_stack.close()
        qk_stack.close()   # qh/kh no longer needed

        # ---- phase D: AV + renorm;  phase E: output projection ------------
        aopool = ctx.enter_context(tc.tile_pool(name="aop", bufs=1))
        aop = [aopool.tile([P, 512], f16, name=f"aop{j}") for j in range(8)]
        with ExitStack() as dctx:
            dpool = dctx.enter_context(tc.tile_pool(name="dpool", bufs=2))
            epool = dctx.enter_context(tc.tile_pool(name="epool", bufs=2))
            dpsum = dctx.enter_context(
                tc.tile_pool(name="dpsum", bufs=2, space="PSUM")
            )
            bpsum2 = dctx.enter_context(
                tc.tile_pool(name="bpsum2", bufs=2, space="PSUM")
            )
            opsum = dctx.enter_context(
                tc.tile_pool(name="opsum", bufs=1, space="PSUM")
            )
            wopool = dctx.enter_context(tc.tile_pool(name="wop", bufs=1))
            # WoT rows packed [d, j, dout] per head-PAIR (d = 128 = two
            # heads' features) so lhsT/rhs are full-K 128-partition tiles.
            wo = wopool.tile([P, 8, D], f16)
            wor_ = WoT[:].rearrange("(j d) o -> d j o", d=128)
            for ch in range(4):
                nc.sync.dma_start(
                    wo[:, ch * 2:(ch + 1) * 2, :],
                    wor_[:, ch * 2:(ch + 1) * 2, :],
                )
            tmpA = [dpool.tile([P, 512], f16, name=f"tmpA{h}", bufs=1)
                    for h in range(16)]
            # D1: s-tiles 0-3 chains for ALL heads (independent of the second
            # Z allreduce) — keeps PE busy through the AR window.
            for h in range(16):
                psA = dpsum.tile([P, 512], f32, tag="avpA")
                for i in range(4):
                    nc.tensor.matmul(
                        psA[0:65, :], vaug[i][:, h, :], wbuf[:, h * 8 + i, :],
                        start=(i == 0), stop=(i == 3),
                    )
                nc.scalar.copy(tmpA[h][0:65, :], psA[0:65, :])
            # D2: s-tiles 4-7 chains, merge, renorm
            for h in range(16):
                psB = dpsum.tile([P, 512], f32, tag="avpB")
                for i in range(4, 8):
                    nc.tensor.matmul(
                        psB[0:65, :], vaug[i][:, h, :], wbuf[:, h * 8 + i, :],
                        start=(i == 4), stop=(i == 7),
                    )
                tmp = dpool.tile([P, 512], f32, tag="rtmp")
                nc.vector.tensor_add(
                    out=tmp[0:65, :], in0=psB[0:65, :], in1=tmpA[h][0:65, :]
                )
                nc.scalar.activation(
                    tmp[64:65, :], tmp[64:65, :], AF.Ln,
                    bias=eps_t[64:65, :],
                )
                rinv = dpool.tile([P, 512], f16, tag="rinv")
                nc.scalar.activation(
                    rinv[64:65, :], tmp[64:65, :], AF.Exp,
                    scale=-1.0,
                )
                pb = bpsum2.tile([64, 512], f32, tag="bcp")
                nc.tensor.matmul(
                    pb[:], ones1[64:65, :], rinv[64:65, :],
                    start=True, stop=True,
                )
                bc = dpool.tile([64, 512], f32, tag="bc")
                nc.vector.tensor_copy(bc[:], pb[:])
                po = (h % 2) * 64
                nc.vector.tensor_mul(
                    out=aop[h // 2][po:po + 64, :],
                    in0=tmp[0:64, :], in1=bc[:],
                )

            # ---- phase E: output projection (interleaves with D).  One
            # LDWEIGHTS per (tch, j) feeds both dout halves. ----
            for tch in range(4):
                pso = opsum.tile([P, 2, 512], f32, tag="outp")
                for j in range(8):
                    lw = aop[j][:, tch * P:(tch + 1) * P]
                    nc.tensor.matmul(
                        pso[:, 0, :], lw, wo[:, j, 0:512],
                        start=(j == 0), stop=(j == 7),
                    )
                    nc.tensor.matmul(
                        pso[:, 1, :], lw, wo[:, j, 512:1024],
                        start=(j == 0), stop=(j == 7),
                    )
                ot = epool.tile([P, 2, 512], f32, tag="ot")
                nc.vector.tensor_copy(ot[:], pso[:])
                nc.sync.dma_start(
                    out[tch * P:(tch + 1) * P, :],
                    ot[:].rearrange("p a b -> p (a b)"),
                )


def build_nc():
    nc = bass.Bass(num_devices=NCORES)
    qT = nc.dram_tensor("qT", [D, TL], f16, kind="ExternalInput")
    kT = nc.dram_tensor("kT", [D, S], f16, kind="ExternalInput")
    vT = nc.dram_tensor("vT", [D, S], f16, kind="ExternalInput")
    WqT = nc.dram_tensor("WqT", [D, D], f16, kind="ExternalInput")
    WkT = nc.dram_tensor("WkT", [D, D], f16, kind="ExternalInput")
    WvT = nc.dram_tensor("WvT", [D, D], f16, kind="ExternalInput")
    WoT = nc.dram_tensor("WoT", [D, D], f16, kind="ExternalInput")
    biasT = nc.dram_tensor("biasT", [8, 4, P, 4, TL], bf16, kind="ExternalInput")
    out = nc.dram_tensor("out", [TL, D], f32, kind="ExternalOutput")
    with tile.TileContext(nc) as tc:
        _emit(nc, tc, qT, kT, vT, WqT, WkT, WvT, WoT, biasT, out)
    _split_multi_waits(nc)
    return nc


_NC_CACHE = {}


def _get_nc():
    if "nc" not in _NC_CACHE:
        _NC_CACHE["nc"] = build_nc()
    return _NC_CACHE["nc"]


# ---------------------------------------------------------------------------
# Host wrapper
# ---------------------------------------------------------------------------
def _prep_inputs(q, k, v, attn_mask, attn_bias, Wq, Wk, Wv, Wo):
    q = np.asarray(q, np.float32)
    k = np.asarray(k, np.float32)
    v = np.asarray(v, np.float32)
    mask = np.asarray(attn_mask)
    bias = np.asarray(attn_bias, np.float32)
    WqT = np.ascontiguousarray(np.asarray(Wq, np.float32).T).astype(np.float16)
    WkT = np.ascontiguousarray(np.asarray(Wk, np.float32).T).astype(np.float16)
    WvT = np.ascontiguousarray(np.asarray(Wv, np.float32).T).astype(np.float16)
    WoT = np.ascontiguousarray(np.asarray(Wo, np.float32).T).astype(np.float16)

    # bias + additive mask, transposed to [h, s, t], tiled for DMA:
    # [s_tile, half, p, head_in_half, t]  with h = half*8+hh, s = s_tile*128+p
    pre = bias + np.where(mask, np.float32(NEG), np.float32(0.0))[None]
    biasT_halves = []
    for th in range(2):
        arrT = pre[:, th * TL:(th + 1) * TL, :].transpose(0, 2, 1)  # [h,s,t]
        arrT = np.ascontiguousarray(arrT).reshape(4, 4, 8, P, TL)
        arrT = arrT.transpose(2, 0, 3, 1, 4)  # [i, qtr, p, hh, t]
        biasT_halves.append(
            np.ascontiguousarray(arrT).astype(ml_dtypes.bfloat16)
        )

    in_maps = []
    kTs, vTs = {}, {}
    for c in range(NCORES):
        b, th = c // 2, c % 2
        if b not in kTs:
            kTs[b] = np.ascontiguousarray(k[b].T).astype(np.float16)
            vTs[b] = np.ascontiguousarray(v[b].T).astype(np.float16)
        qTc = np.ascontiguousarray(
            q[b].T[:, th * TL:(th + 1) * TL]).astype(np.float16)
        in_maps.append({
            "qT": qTc, "kT": kTs[b], "vT": vTs[b],
            "WqT": WqT, "WkT": WkT, "WvT": WvT, "WoT": WoT,
            "biasT": biasT_halves[th],
        })
    return in_maps


def run(inputs, trace=False):
    nc = _get_nc()
    in_maps = _prep_inputs(**inputs)
    res = run_bass_kernel_spmd(
        nc, in_maps, core_ids=list(range(NCORES)), trace=trace,
    )
    full = np.empty((B, T, D), np.float32)
    for c in range(NCORES):
        b, th = c // 2, c % 2
        full[b, th * TL:(th + 1) * TL, :] = res.results[c]["out"]
    return full, res


def kernel(**inputs) -> np.ndarray:
    full, _ = run(inputs, trace=False)
    return full
